# revision 30
# baseline (speedup 1.0000x reference)
"""CRF forward-algorithm loss on 8 Trainium2 NeuronCores.

Math: the reference does, per step t (8192 steps, K=2048 tags):
    fv'[n] = logsumexp_p(fv[p] + T[n,p]) + h[t,n]
and finally logsumexp(fv + T[END]).

We run the recurrence in LINEAR space: with w ~ exp(fv) (rescaled each step),
    m[n]  = sum_p expT[n,p] * w[p]          (matvec, tensor engine, bf16)
    S     = sum_n m[n]                      (computed as an extra matvec column
                                             carrying colsum(expT))
    w'[n] = (m[n] / S) * exp(h[t,n])
    record 1/S
answer = log(sum_p expT[END,p] * w_final[p]) - sum_t log(recip_t)

Distribution: tensor-parallel over the `next` axis. Each core holds the
[2048 prev x 256 next] slice of expT^T as the matmul MOVING operand (resident
in SBUF, bf16) and computes a 256-slice of m per step; slices are exchanged
every step with direct core-to-core SBUF remote DMA broadcasts (XOR slot
scheme keeps the program SPMD-uniform: receiver r's gather slot k holds data
from core r^k; all per-core tensors are laid out by the host accordingly).

Per-core, per-step pipeline (raw bass, hand-scheduled, monotonic semaphores):
  PE   : 16 accumulating matmuls [128c x 1] x [128c x 257] -> psum row [1,257]
         + 2 tiny matmuls that transpose the scaled row into [128,2]
  DVE  : 1/S, scale row, copy transposed cols to send tile + own gather slot,
         w' = gather * expH  (bf16)
  ACT  : expH = exp(h-tile)  ([128,16], streamed from HBM in blocks)
  GPSIMD: 7 remote_dma_broadcast preps + trigger (data), credit broadcast
  SP   : h block prefetch DMA
"""

import sys

if "/opt/trn_rl_repo" not in sys.path:
    sys.path.insert(0, "/opt/trn_rl_repo")

import numpy as np
import ml_dtypes

import concourse.bass as bass
import concourse.bacc as bacc
import concourse.mybir as mybir

START_IDX = 0
END_IDX = 1
K = 2048
SEQ = 8192
NCORES = 8
P = 128
SLICE = K // NCORES          # 256 nexts per core
MT = K // P                  # 16 contract chunks of 128
MCOLS = SLICE + 1            # 256 nexts + 1 colsum column
BF16 = mybir.dt.bfloat16
F32 = mybir.dt.float32
NPBF16 = ml_dtypes.bfloat16


def build_bass(seq_blocks: int, blk_steps: int, dbg_delay: int = 0) -> bass.Bass:
    """Device program. seq = seq_blocks * blk_steps, blk_steps must be even."""
    assert blk_steps % 2 == 0
    seq = seq_blocks * blk_steps
    nc = bacc.Bacc(None, target_bir_lowering=False, num_devices=NCORES)

    movq = nc.declare_dram_parameter("movq", [P, MT * MCOLS], BF16, isOutput=False)
    hq = nc.declare_dram_parameter("hq", [P, seq * MT], F32, isOutput=False)
    winit = nc.declare_dram_parameter("winit", [P, MT], BF16, isOutput=False)
    wout = nc.declare_dram_parameter("wout", [P, MT], BF16, isOutput=True)
    rec_out = nc.declare_dram_parameter("rec", [1, seq], F32, isOutput=True)

    movsb = nc.alloc_sbuf_tensor("movsb", [P, MT * MCOLS], BF16)
    w_sb = nc.alloc_sbuf_tensor("w_sb", [P, MT], BF16)
    hq_sb = nc.alloc_sbuf_tensor("hq_sb", [P, 2 * blk_steps * MT], F32)  # holds exp(h)
    graw = nc.alloc_sbuf_tensor("graw", [P, 2 * MT], F32)      # parity halves
    sendt = nc.alloc_sbuf_tensor("sendt", [P, 4], F32)         # parity 2+2
    mrow = nc.alloc_sbuf_tensor("mrow", [1, SLICE], F32)
    one_sb = nc.alloc_sbuf_tensor("one_sb", [1, 1], F32)
    rec_sb = nc.alloc_sbuf_tensor("rec_sb", [1, seq], F32)

    psum_m = nc.alloc_psum_tensor("psum_m", [P, 512], F32)     # row 0 used
    psum_ta = nc.alloc_psum_tensor("psum_ta", [P, 512], F32)   # col 0 used
    psum_tb = nc.alloc_psum_tensor("psum_tb", [P, 512], F32)

    # semaphores
    sem_mm = nc.alloc_semaphore("sem_mm")        # PE matvec done     +1/step
    sem_row = nc.alloc_semaphore("sem_row")      # scaled row ready   +1/step
    sem_tp = nc.alloc_semaphore("sem_tp")        # transposes done    +1/step
    sem_send = nc.alloc_semaphore("sem_send")    # send tile ready    +1/step
    sem_wdone = nc.alloc_semaphore("sem_wdone")  # w' ready           +1/step
    rsem = [nc.alloc_semaphore(f"rsem{i}") for i in range(2)]   # +14/same-parity step (remote)
    csem = [nc.alloc_semaphore(f"csem{i}") for i in range(2)]   # +14/same-parity step (remote)
    lsem = nc.alloc_semaphore("lsem")            # data send local    +112/step
    lsem_c = nc.alloc_semaphore("lsem_c")        # credit send local  +16/step
    psem_d = nc.alloc_semaphore("psem_d")        # data descs written +7/step
    psem_c = nc.alloc_semaphore("psem_c")        # credit descs       +1/step
    dma0 = nc.alloc_semaphore("dma0")            # prologue loads
    hqsem = [nc.alloc_semaphore(f"hqsem{i}") for i in range(2)]  # h DMAs, +16/same-parity block

    pe, dve, act, gp, sp = nc.tensor, nc.vector, nc.scalar, nc.gpsimd, nc.sync

    # ---- prologue ----
    gp.memset(one_sb[:, :], 1.0)
    sp.dma_start(out=movsb[:, :], in_=movq[:, :]).then_inc(dma0, 16)
    sp.dma_start(out=w_sb[:, :], in_=winit[:, :]).then_inc(dma0, 16)
    sp.dma_start(out=hq_sb[:, 0 : blk_steps * MT], in_=hq[:, 0 : blk_steps * MT]).then_inc(hqsem[0], 16)
    if seq_blocks > 1:
        sp.dma_start(
            out=hq_sb[:, blk_steps * MT : 2 * blk_steps * MT],
            in_=hq[:, blk_steps * MT : 2 * blk_steps * MT],
        ).then_inc(hqsem[1], 16)
    pe.wait_ge(dma0, 32)
    # no remote traffic may be emitted before every core has loaded + zeroed state
    nc.all_core_barrier()

    # ---- per-engine monotonic threshold registers ----
    def reg(engine, name, val=0):
        r = engine.alloc_register(name)
        engine.reg_mov(r, val)
        return r

    pe_wd = reg(pe, "pe_wd")
    pe_row = reg(pe, "pe_row")
    v_mm = reg(dve, "v_mm")
    v_tp = reg(dve, "v_tp")
    v_ls = reg(dve, "v_ls")
    v_rs = [reg(dve, f"v_rs{i}") for i in range(2)]
    v_rec = reg(dve, "v_rec")      # record write offset (elements)
    v_hq = reg(dve, "v_hq")        # exp(h) tile read offset
    v_hqs = reg(dve, "v_hqs")      # hq block threshold
    g_send = reg(gp, "g_send")
    g_cs = [reg(gp, f"g_cs{i}") for i in range(2)]
    g_wd = reg(gp, "g_wd")
    g_pd = reg(gp, "g_pd")
    g_pc = reg(gp, "g_pc")
    s_src = reg(sp, "s_src")
    s_tmp = reg(sp, "s_tmp")
    s_cond = reg(sp, "s_cond")

    def emit_step(par: int):
        # ---------------- PE ----------------
        pe.wait_ge(sem_wdone, pe_wd)
        pe.reg_add(pe_wd, pe_wd, 1)
        for j2 in range(MT):
            pe.matmul(
                psum_m[0:1, 0:MCOLS],
                w_sb[:, j2 : j2 + 1],
                movsb[:, j2 * MCOLS : (j2 + 1) * MCOLS],
                start=(j2 == 0),
                stop=(j2 == MT - 1),
            ).then_maybe_inc((sem_mm, 1) if j2 == MT - 1 else None)
        pe.reg_add(pe_row, pe_row, 1)
        pe.wait_ge(sem_row, pe_row)
        pe.matmul(psum_ta[0:P, 0:1], mrow[0:1, 0:P], one_sb[0:1, 0:1], start=True, stop=True)
        pe.matmul(psum_tb[0:P, 0:1], mrow[0:1, P : 2 * P], one_sb[0:1, 0:1], start=True, stop=True).then_inc(sem_tp, 1)

        # ---------------- DVE ----------------
        dve.reg_add(v_mm, v_mm, 1)
        dve.wait_ge(sem_mm, v_mm)
        rec_ap = bass.AP(rec_sb, v_rec, [[seq, 1], [1, 1]])
        dve.reciprocal(rec_ap, psum_m[0:1, SLICE : SLICE + 1])
        dve.drain()
        dve.tensor_scalar(
            mrow[0:1, 0:SLICE], psum_m[0:1, 0:SLICE], rec_ap, None, op0=mybir.AluOpType.mult
        ).then_inc(sem_row, 1)
        dve.reg_add(v_rec, v_rec, 1)
        dve.reg_add(v_tp, v_tp, 1)
        dve.wait_ge(sem_tp, v_tp)
        dve.wait_ge(lsem, v_ls)          # sends of step t-1 fully read sendt
        dve.reg_add(v_ls, v_ls, 112)
        dve.tensor_copy(sendt[:, 2 * par : 2 * par + 1], psum_ta[0:P, 0:1])
        dve.tensor_copy(sendt[:, 2 * par + 1 : 2 * par + 2], psum_tb[0:P, 0:1]).then_inc(sem_send, 1)
        # own slice -> gather slot 0 of this parity (read from psum, not sendt,
        # to avoid a same-engine RAW on sendt)
        dve.tensor_copy(graw[:, MT * par : MT * par + 1], psum_ta[0:P, 0:1])
        dve.tensor_copy(graw[:, MT * par + 1 : MT * par + 2], psum_tb[0:P, 0:1])
        dve.reg_add(v_rs[par], v_rs[par], 14)
        dve.wait_ge(rsem[par], v_rs[par])
        if dbg_delay:
            dve.nop(cycle_cnt=dbg_delay)
        dve.drain()
        dve.tensor_tensor(
            w_sb[:, :],
            graw[:, MT * par : MT * (par + 1)],
            bass.AP(hq_sb, v_hq, [[2 * blk_steps * MT, P], [1, MT]]),
            op=mybir.AluOpType.mult,
        ).then_inc(sem_wdone, 1)
        dve.reg_add(v_hq, v_hq, MT)

        # ---------------- GPSIMD ----------------
        for k in range(1, NCORES):
            rd = [None] * NCORES
            rd[k] = (0, k)
            gp.remote_dma_broadcast(
                out_ap=graw[:, MT * par + 2 * k : MT * par + 2 * k + 2],
                in_ap=sendt[:, 2 * par : 2 * par + 2],
                remote_sem=rsem[par],
                local_sem=lsem,
                rdests=rd,
            ).then_inc(psem_d, 1)
        gp.reg_add(g_send, g_send, 1)
        gp.wait_ge(sem_send, g_send)
        gp.reg_add(g_pd, g_pd, 7)
        gp.wait_ge(psem_d, g_pd)
        gp.wait_ge(csem[par], g_cs[par])  # receivers consumed graw[par] at t-2
        gp.reg_add(g_cs[par], g_cs[par], 14)
        gp.trigger_dma(count=7)
        gp.reg_add(g_wd, g_wd, 1)
        gp.wait_ge(sem_wdone, g_wd)
        rd = [None] * NCORES
        for k in range(1, NCORES):
            rd[k] = (0, k)
        gp.remote_sem_update_broadcast(
            remote_sem=csem[par], local_sem=lsem_c, rdests=rd
        ).then_inc(psem_c, 1)
        gp.reg_add(g_pc, g_pc, 1)
        gp.wait_ge(psem_c, g_pc)
        gp.trigger_dma(count=1)

    # ---- main loop: superblocks of two h-blocks (static buffer parity) ----
    assert seq_blocks % 2 == 0
    with nc.Fori(0, seq_blocks // 2) as g:
        for p01 in range(2):  # h-block index blk = 2*g + p01, buffer half p01
            # DVE: gate on this block's h DMA; read offset = p01 half
            dve.reg_alu(v_hqs, g, 16, op=mybir.AluOpType.mult)
            dve.reg_add(v_hqs, v_hqs, 16)
            dve.wait_ge(hqsem[p01], v_hqs)
            dve.reg_mov(v_hq, p01 * blk_steps * MT)

            # SP: once DVE finishes block 2g+p01, prefetch block 2g+2+p01
            if seq_blocks > 2:
                sp.reg_alu(s_tmp, g, 2 * blk_steps, op=mybir.AluOpType.mult)
                sp.reg_add(s_tmp, s_tmp, (1 + p01) * blk_steps)
                sp.reg_mov(s_cond, 0)
                sp.reg_add(s_cond, g, 0)
                with sp.If_lt(s_cond, seq_blocks // 2 - 1):
                    sp.wait_ge(sem_wdone, s_tmp)
                    sp.reg_add(s_src, g, 0)
                    sp.reg_alu(s_src, s_src, 2 * blk_steps * MT, op=mybir.AluOpType.mult)
                    sp.reg_add(s_src, s_src, (2 + p01) * blk_steps * MT)
                    sp.dma_start(
                        out=bass.AP(
                            hq_sb,
                            p01 * blk_steps * MT,
                            [[2 * blk_steps * MT, P], [1, blk_steps * MT]],
                        ),
                        in_=bass.AP(hq, s_src, [[seq * MT, P], [1, blk_steps * MT]]),
                    ).then_inc(hqsem[p01], 16)

            with nc.Fori(0, blk_steps // 2):
                emit_step(0)
                emit_step(1)

    # ---- epilogue ----
    sp.wait_ge(sem_row, seq)
    sp.dma_start(out=rec_out[:, :], in_=rec_sb[:, :]).then_inc(dma0, 16)
    sp.wait_ge(sem_wdone, seq)
    sp.dma_start(out=wout[:, :], in_=w_sb[:, :]).then_inc(dma0, 16)
    sp.wait_ge(dma0, 64)
    gp.wait_ge(lsem, 112 * seq)
    gp.wait_ge(lsem_c, 16 * seq)
    nc.all_core_barrier()
    nc.finalize()
    return nc


IDENT_SIGMA = list(range(NCORES))


def probe_slot_map() -> list[int]:
    """Measure the physical slot->sender permutation of remote_dma_broadcast.

    Receiver r's gather slot k receives from logical core (r ^ sigma[k]),
    where sigma depends on the driver's logical->physical NC mapping (e.g. a
    die-1 pair swap makes sigma = [0,1,2,3,6,7,4,5]). One tiny broadcast
    round measures it.
    """
    from concourse.bass_utils import run_bass_kernel_spmd

    nc = bacc.Bacc(None, target_bir_lowering=False, num_devices=NCORES)
    src = nc.declare_dram_parameter("src", [P, 1], F32, isOutput=False)
    dst = nc.declare_dram_parameter("dst", [1, NCORES], F32, isOutput=True)
    sendt = nc.alloc_sbuf_tensor("sendt", [P, 1], F32)
    graw = nc.alloc_sbuf_tensor("graw", [P, NCORES], F32)
    rs = nc.alloc_semaphore("rs")
    ls = nc.alloc_semaphore("ls")
    ps = nc.alloc_semaphore("ps")
    d0 = nc.alloc_semaphore("d0")
    d2 = nc.alloc_semaphore("d2")
    gp, sp, dve = nc.gpsimd, nc.sync, nc.vector
    sp.dma_start(out=sendt[:, :], in_=src[:, :]).then_inc(d0, 16)
    dve.memset(graw[:, :], -1.0)
    nc.all_core_barrier()
    for k in range(1, NCORES):
        rd = [None] * NCORES
        rd[k] = (0, k)
        gp.remote_dma_broadcast(
            out_ap=graw[:, k : k + 1], in_ap=sendt[:, :],
            remote_sem=rs, local_sem=ls, rdests=rd,
        ).then_inc(ps, 1)
    gp.wait_ge(ps, 7)
    gp.wait_ge(d0, 16)
    gp.trigger_dma(count=7)
    dve.tensor_copy(graw[:, 0:1], sendt[:, :])
    sp.wait_ge(rs, 14)
    sp.dma_start(out=dst[:, :], in_=graw[0:1, :]).then_inc(d2, 16)
    sp.wait_ge(d2, 16)
    nc.all_core_barrier()
    nc.finalize()

    in_maps = [{"src": np.full((P, 1), float(r), np.float32)} for r in range(NCORES)]
    res = run_bass_kernel_spmd(nc, in_maps, core_ids=list(range(NCORES)))
    sigma = None
    for r in range(NCORES):
        senders = res.results[r]["dst"].reshape(-1).astype(int)
        sig_r = [int(senders[k]) ^ r for k in range(NCORES)]
        assert sorted(sig_r) == list(range(NCORES)), f"core {r}: bad slot map {senders}"
        if sigma is None:
            sigma = sig_r
        else:
            assert sigma == sig_r, f"inconsistent slot maps {sigma} vs {sig_r}"
    assert sigma is not None and sigma[0] == 0
    return sigma


def prep_inputs(h: np.ndarray, transitions: np.ndarray, seq: int, sigma=None):
    """Host-side layout of per-core inputs (XOR slot scheme).

    Global prev index for core r at matvec position j (j = col*128 + q):
        g_r(j) = 256*(r ^ sigma[j >> 8]) + (j & 255)
    """
    if sigma is None:
        sigma = IDENT_SIGMA
    sig = np.asarray(sigma, dtype=np.int64)
    h32 = np.ascontiguousarray(h.astype(np.float32)[:seq])
    expT = np.exp(transitions.astype(np.float32))
    expTq = expT.astype(NPBF16)
    colsum = expTq.astype(np.float32).sum(axis=0).astype(NPBF16)

    j = np.arange(K)
    # h in [q, t, c] layout for r=0, identity: hq0[q, t, c] = h[t, c*128 + q]
    hq0 = np.ascontiguousarray(
        h32.reshape(seq, MT, P).transpose(2, 0, 1)
    )  # [128, seq, 16]

    in_maps = []
    for r in range(NCORES):
        perm = 256 * (r ^ sig[j >> 8]) + (j & 255)
        # moving tiles: mov[q, j2*257 + col] ; col<256 -> expTq[256r+col, g_r(j2*128+q)]
        A = expTq[256 * r : 256 * (r + 1), :][:, perm]          # [256 next, 2048 j]
        B = np.ascontiguousarray(A.reshape(SLICE, MT, P).transpose(2, 1, 0))  # [q, j2, col]
        C = colsum[perm].reshape(MT, P).T                        # [q, j2]
        mov = np.concatenate([B, C[:, :, None]], axis=2)         # [128, 16, 257]
        mov = np.ascontiguousarray(mov.reshape(P, MT * MCOLS))

        # exp(h): permute the 16 c-columns of hq0:
        # col (2k+half) <- base col (2*(r^sigma[k])+half)
        cperm = np.empty(MT, dtype=np.int64)
        for k in range(NCORES):
            cperm[2 * k] = 2 * (r ^ sigma[k])
            cperm[2 * k + 1] = 2 * (r ^ sigma[k]) + 1
        hqr = np.ascontiguousarray(np.exp(hq0[:, :, cperm]).reshape(P, seq * MT))

        wi = np.zeros((P, MT), dtype=NPBF16)
        # global prev START=0 sits at (q=0, col=2*k0) where sigma[k0] == r
        wi[0, 2 * sigma.index(r)] = 1.0

        in_maps.append({"movq": mov, "hq": hqr, "winit": wi})
    return in_maps, expT


def finalize(results, transitions, seq: int, sigma=None):
    """Combine device outputs into the scalar answer (host, fp64)."""
    if sigma is None:
        sigma = IDENT_SIGMA
    sig = np.asarray(sigma, dtype=np.int64)
    rec = results[0]["rec"].reshape(-1).astype(np.float64)       # recip values
    wfin = results[0]["wout"].astype(np.float64)                 # [128, 16] on core 0
    j = np.arange(K)
    g0 = 256 * sig[j >> 8] + (j & 255)                           # global idx at position j
    w_full = np.empty(K, np.float64)
    w_full[g0] = wfin.T.reshape(-1)                              # position j = c*128+q
    exp_end = np.exp(transitions[END_IDX].astype(np.float64))
    mterm = float(np.dot(exp_end, w_full))
    ans = np.log(mterm) - np.sum(np.log(rec))
    return np.float32(ans)


_SIGMA_CACHE: list[int] | None = None


def get_sigma() -> list[int]:
    global _SIGMA_CACHE
    if _SIGMA_CACHE is None:
        _SIGMA_CACHE = probe_slot_map()
    return _SIGMA_CACHE


def kernel(h: np.ndarray, transitions: np.ndarray) -> np.ndarray:
    from concourse.bass_utils import run_bass_kernel_spmd

    sigma = get_sigma()
    seq_blocks, blk_steps = 16, SEQ // 16
    nc = build_bass(seq_blocks, blk_steps)
    in_maps, _ = prep_inputs(np.asarray(h), np.asarray(transitions), SEQ, sigma)
    res = run_bass_kernel_spmd(nc, in_maps, core_ids=list(range(NCORES)))
    return finalize(res.results, np.asarray(transitions), SEQ, sigma)


if __name__ == "__main__":
    import reference

    inputs = {k: np.asarray(v) for k, v in reference.setup_inputs().items()}
    out = kernel(**inputs)
    print("kernel:", out)
